# revision 5
# baseline (speedup 1.0000x reference)
"""Int4-packed linear (group-quantized, 256-group) on 8 Trainium2 cores.

Column-parallel: each core owns 1024 of 8192 out_features.

Math per core (out^T orientation, o on partitions):
  out[o, t] = sum_g s[o,g] * R_g[o,t] + corr[o,t]
  R_g[o,t]  = sum_{i in g} q[o,i] * x[t,i]        (q in 0..15)
  corr[o,t] = -8*sum_g s[o,g]*xsum_g[t] + bias[o]   (precomputed on host)

Weights ship as fp8e4m3 nibble planes (exact small integers), x as bf16.
The weight stream (8 MiB/core, o-tile-major contiguous reads) rides the
Sync HWDGE ring; x + scales/corr + output stores ride the Scalar HWDGE
ring so the two rings overlap. The stream runs at the per-core HBM
roofline, so the kernel is stream-bound in the middle; the tail after
the last weight byte is minimized by pipelining the final o-tile in
four shrinking pieces (16/12/3/1 groups).

Combine per o-tile, split across DVE/ACT/Pool (only DVE and ACT may
read PSUM):
  DVE   : prod[o,t,0:16]   = PSUM_A[o,g,t] * s[o,g]  (transposing read)
  ACT   : prod[o,t,16:32]  = PSUM_B[o,g,t]           (transposing copy)
  Pool  : prodB2[o,t,0:16] = prod[o,t,16:32] * s[o,g]
  Pool  : half[o,t,0:16]   = prod[o,t,0:16] + prodB2
  ACT   : half[o,t,16]     = corr (staged early, off-chain)
  DVE   : y[o,t] = reduce_X(half[o,t,0:17])
"""

import sys

import numpy as np
import ml_dtypes

sys.path.insert(0, "/opt/trn_rl_repo")

import concourse.bass as bass  # noqa: E402
import concourse.mybir as mybir  # noqa: E402
import concourse.tile as tile  # noqa: E402
from concourse import bacc  # noqa: E402

NCORES = 8
TOKENS = 64
IN_F = 8192
OUT_F = 8192
GROUP = 256
OC = OUT_F // NCORES  # 1024 out-features per core
NCHUNK = IN_F // 128  # 64 K-chunks of 128
NG = IN_F // GROUP  # 32 groups
NH = NG // 2  # 16 groups per PSUM half tile
NOT = OC // 128  # 8 o-tiles per core

_cache = {}


def _build_nc():
    if "nc" in _cache:
        return _cache["nc"], _cache["names"]

    f32 = mybir.dt.float32
    bf16 = mybir.dt.bfloat16
    nc = bacc.Bacc(None, target_bir_lowering=False, debug=False)
    with tile.TileContext(nc) as tc:
        with tc.tile_pool(name="dram", bufs=1, space="DRAM") as dram:
            # w8[ot, p, r, c] = nibble_fp8[ot*128 + c, 128*r + p]
            # (o-tile-major: each o-tile's weights are one contiguous 1 MiB)
            w8 = dram.tile([NOT, 128, NCHUNK, 128], mybir.dt.float8e4,
                           kind="ExternalInput")
            xt = dram.tile([128, NCHUNK, TOKENS], bf16, kind="ExternalInput")
            # scco[:, ot, 0:32] = scales, scco[:, ot, 32:96] = corr
            scco = dram.tile([128, NOT, NG + TOKENS], bf16,
                             kind="ExternalInput")
            outT = dram.tile([128, NOT, TOKENS], bf16, kind="ExternalOutput")

            with (
                tc.tile_pool(name="wsb", bufs=1) as wsb,
                tc.tile_pool(name="xsb", bufs=1) as xsb,
                tc.tile_pool(name="small", bufs=1) as small,
                tc.tile_pool(name="cmb", bufs=1) as cmb,
                tc.tile_pool(name="ps", bufs=4, space="PSUM") as ps,
            ):
                w_all = wsb.tile([128, NOT, NCHUNK, 128], mybir.dt.float8e4)
                x_all = xsb.tile([128, NCHUNK, TOKENS], bf16)
                scco_sb = small.tile([128, NOT, NG + TOKENS], bf16)
                prod_all = cmb.tile([128, NOT - 1, TOKENS, NG], bf16)
                prodB2_all = cmb.tile([128, NOT - 1, TOKENS, NH], bf16,
                                      tag="pb2")
                half_all = cmb.tile([128, NOT - 1, TOKENS, NH + 1], bf16,
                                    tag="hf")
                # o-tile 7 slots: 0:16 = g0-15, 16 = corr, 17:29 = g16-27,
                # 29:32 = g28-30, 32 = g31
                prod7 = cmb.tile([128, TOKENS, NG + 1], bf16, tag="p7")
                prodB27 = cmb.tile([128, TOKENS, 12], bf16, tag="pb27")
                part7 = cmb.tile([128, 4, TOKENS], bf16, tag="part")
                y_all = cmb.tile([128, NOT, TOKENS], bf16, tag="y")

                # x + scales/corr on the Scalar HWDGE ring; the 8 MiB weight
                # stream owns the Sync ring. Both rings drain through the
                # same 16 SDMA engines, so aggregate rate caps at the HBM
                # per-core limit while per-ring receipt latency overlaps.
                nc.scalar.dma_start(out=x_all[:], in_=xt[:])
                nc.scalar.dma_start(out=scco_sb[:], in_=scco[:])

                for ot in range(NOT - 1):
                    nc.sync.dma_start(out=w_all[:, ot, :, :], in_=w8[ot])
                # last o-tile in 4 shrinking pieces so the post-stream
                # chain is short: g0-15 / g16-27 / g28-30 / g31
                for c0, c1 in ((0, 32), (32, 56), (56, 62), (62, 64)):
                    nc.sync.dma_start(out=w_all[:, NOT - 1, c0:c1, :],
                                      in_=w8[NOT - 1, :, c0:c1, :])

                def mm_groups(ot, glo, ghi, r_ps):
                    # start once per 2 KB PSUM bank (8 slots of 64 fp32);
                    # the bank-wide pending-zero covers every slot's first
                    # write. stop on the last matmul touching each bank.
                    n = ghi - glo
                    for gg in range(n):
                        g = glo + gg
                        nc.tensor.matmul(
                            r_ps[:, gg, :],
                            lhsT=w_all[:, ot, 2 * g, :],
                            rhs=x_all[:, 2 * g, :],
                            start=(gg % 8 == 0),
                            stop=False,
                        )
                        nc.tensor.matmul(
                            r_ps[:, gg, :],
                            lhsT=w_all[:, ot, 2 * g + 1, :],
                            rhs=x_all[:, 2 * g + 1, :],
                            start=False,
                            stop=(gg == n - 1 or gg % 8 == 7),
                        )

                def psum_tg(r_ps, npg):
                    # PSUM [o, g, t] viewed as [o, t, g] (strided g-read,
                    # contiguous SBUF write)
                    return bass.AP(
                        tensor=r_ps.tensor,
                        offset=r_ps.offset,
                        ap=[r_ps.ap[0], [1, TOKENS], [TOKENS, npg]],
                    )

                def s_bc(ot, lo, n):
                    # s[o, g] broadcast along t
                    s_ot = scco_sb[:, ot, :]
                    return bass.AP(
                        tensor=s_ot.tensor,
                        offset=s_ot.offset + lo,
                        ap=[s_ot.ap[0], [0, TOKENS], [1, n]],
                    )

                mul = mybir.AluOpType.mult
                add = mybir.AluOpType.add

                with nc.allow_low_precision("bf16 combine, validated vs 2e-2"):
                    # stage corr early (ACT, off the combine chain)
                    for ot in range(NOT - 1):
                        nc.scalar.copy(out=half_all[:, ot, :, NH],
                                       in_=scco_sb[:, ot, NG:])
                    nc.scalar.copy(out=prod7[:, :, NH],
                                   in_=scco_sb[:, NOT - 1, NG:])

                    for ot in range(NOT - 1):
                        rA = ps.tile([128, NH, TOKENS], f32, tag="rA", bufs=2)
                        mm_groups(ot, 0, NH, rA)
                        rB = ps.tile([128, NH, TOKENS], f32, tag="rB", bufs=2)
                        mm_groups(ot, NH, NG, rB)
                        prod = prod_all[:, ot, :, :]
                        prodB2 = prodB2_all[:, ot, :, :]
                        half = half_all[:, ot, :, :]
                        # DVE: A half scaled straight out of PSUM
                        nc.vector.tensor_tensor(
                            out=prod[:, :, 0:NH], in0=psum_tg(rA, NH),
                            in1=s_bc(ot, 0, NH), op=mul)
                        # ACT: B half raw transposing copy out of PSUM
                        nc.scalar.copy(out=prod[:, :, NH:NG],
                                       in_=psum_tg(rB, NH))
                        # Pool: scale B half, then fold A+B
                        nc.gpsimd.tensor_tensor(
                            out=prodB2[:], in0=prod[:, :, NH:NG],
                            in1=s_bc(ot, NH, NH), op=mul)
                        nc.gpsimd.tensor_tensor(
                            out=half[:, :, 0:NH], in0=prod[:, :, 0:NH],
                            in1=prodB2[:], op=add)
                        # DVE: reduce 16 groups + corr
                        nc.vector.tensor_reduce(
                            out=y_all[:, ot, :], in_=half,
                            axis=mybir.AxisListType.X, op=add)

                    # ship o-tiles 0-6 as one store on the Scalar ring
                    nc.scalar.dma_start(out=outT[:, 0:NOT - 1, :],
                                        in_=y_all[:, 0:NOT - 1, :])

                    # ---- o-tile 7, pipelined in 4 pieces ----
                    ot = NOT - 1
                    # piece A: g0-15 -> slots 0:16, reduce with corr (slot 16)
                    rA = ps.tile([128, NH, TOKENS], f32, tag="rA", bufs=2)
                    mm_groups(ot, 0, 16, rA)
                    nc.vector.tensor_tensor(
                        out=prod7[:, :, 0:16], in0=psum_tg(rA, 16),
                        in1=s_bc(ot, 0, 16), op=mul)
                    nc.vector.tensor_reduce(
                        out=part7[:, 0, :], in_=prod7[:, :, 0:17],
                        axis=mybir.AxisListType.X, op=add)
                    # piece B: g16-27 -> slots 17:29 (ACT copy, Pool scale)
                    rB = ps.tile([128, 12, TOKENS], f32, tag="rB", bufs=2)
                    mm_groups(ot, 16, 28, rB)
                    nc.scalar.copy(out=prod7[:, :, 17:29],
                                   in_=psum_tg(rB, 12))
                    nc.gpsimd.tensor_tensor(
                        out=prodB27[:], in0=prod7[:, :, 17:29],
                        in1=s_bc(ot, 16, 12), op=mul)
                    nc.vector.tensor_reduce(
                        out=part7[:, 1, :], in_=prodB27,
                        axis=mybir.AxisListType.X, op=add)
                    nc.vector.tensor_tensor(
                        out=part7[:, 2, :], in0=part7[:, 0, :],
                        in1=part7[:, 1, :], op=add)
                    # piece C: g28-30 -> slots 29:32 (DVE, scaled from PSUM)
                    rC = ps.tile([128, 3, TOKENS], f32, tag="rA", bufs=2)
                    mm_groups(ot, 28, 31, rC)
                    nc.vector.tensor_tensor(
                        out=prod7[:, :, 29:32], in0=psum_tg(rC, 3),
                        in1=s_bc(ot, 28, 3), op=mul)
                    nc.vector.tensor_reduce(
                        out=part7[:, 1, :], in_=prod7[:, :, 29:32],
                        axis=mybir.AxisListType.X, op=add)
                    nc.vector.tensor_tensor(
                        out=part7[:, 3, :], in0=part7[:, 2, :],
                        in1=part7[:, 1, :], op=add)
                    # piece D: g31 -> slot 32; token-split final add + store
                    rD = ps.tile([128, 1, TOKENS], f32, tag="rB", bufs=2)
                    mm_groups(ot, 31, 32, rD)
                    nc.vector.tensor_tensor(
                        out=prod7[:, :, 32], in0=psum_tg(rD, 1),
                        in1=s_bc(ot, 31, 1), op=mul)
                    for t0, t1 in ((0, TOKENS // 2), (TOKENS // 2, TOKENS)):
                        nc.vector.tensor_tensor(
                            out=y_all[:, ot, t0:t1],
                            in0=part7[:, 3, t0:t1],
                            in1=prod7[:, t0:t1, 32], op=add)
                        nc.scalar.dma_start(out=outT[:, ot, t0:t1],
                                            in_=y_all[:, ot, t0:t1])

    nc.compile()
    names = dict(w8=w8.name, xt=xt.name, scco=scco.name, outT=outT.name)
    _cache["nc"] = nc
    _cache["names"] = names
    return nc, names


def _gather_core(outT_host):
    # outT_host[p, ot, t] -> [t, ot*128 + p]
    o = np.asarray(outT_host).astype(np.float32)
    return o.transpose(2, 1, 0).reshape(TOKENS, OC)


def _host_prep(x, weight_packed, scales, bias):
    """Build the 8 per-core input maps."""
    _, names = _build_nc()

    wp = np.ascontiguousarray(weight_packed).view(np.uint32)  # [8192, 1024]
    shifts = (np.arange(8, dtype=np.uint32) * 4)[None, None, :]
    nib = ((wp[:, :, None] >> shifts) & np.uint32(0xF)).astype(np.uint8)
    nib = nib.reshape(OUT_F, IN_F)  # n[o, i]
    lut = np.arange(16, dtype=np.float32).astype(ml_dtypes.float8_e4m3)
    nfp8 = lut[nib]  # [8192, 8192] fp8, exact

    xb = x.astype(ml_dtypes.bfloat16)
    xf = xb.astype(np.float32)
    # xt_host[p, r, t] = x_bf16[t, 128r + p]
    xt_host = np.ascontiguousarray(
        xb.T.reshape(NCHUNK, 128, TOKENS).transpose(1, 0, 2))
    # corr[o, t] = -8 * sum_g s[o,g] * xsum_g[t] + bias[o]
    xsum = xf.reshape(TOKENS, NG, GROUP).sum(axis=2)  # [t, g]
    corr = (-8.0 * scales.astype(np.float64) @ xsum.astype(np.float64).T
            + bias.astype(np.float64)[:, None]).astype(np.float32)  # [8192, 64]

    in_maps = []
    for k in range(NCORES):
        osl = slice(OC * k, OC * (k + 1))
        nk = nfp8[osl]  # [1024, 8192]
        # w8_host[ot, p, r, c] = nk[ot*128 + c, 128*r + p]
        w8_host = np.ascontiguousarray(
            nk.reshape(NOT, 128, NCHUNK, 128).transpose(0, 3, 2, 1)
        )
        sck = scales[osl]  # [1024, 32]
        scco_host = np.empty((128, NOT, NG + TOKENS), dtype=ml_dtypes.bfloat16)
        scco_host[:, :, :NG] = sck.reshape(NOT, 128, NG).transpose(1, 0, 2)
        scco_host[:, :, NG:] = corr[osl].reshape(NOT, 128, TOKENS).transpose(
            1, 0, 2)
        in_maps.append({
            names["w8"]: w8_host,
            names["xt"]: xt_host,
            names["scco"]: np.ascontiguousarray(scco_host),
        })
    return in_maps


def kernel(x, weight_packed, scales, bias):
    from concourse.bass_utils import run_bass_kernel_spmd

    nc, names = _build_nc()
    in_maps = _host_prep(x, weight_packed, scales, bias)
    res = run_bass_kernel_spmd(nc, in_maps, core_ids=list(range(NCORES)))
    out = np.concatenate(
        [_gather_core(res.results[k][names["outT"]]) for k in range(NCORES)],
        axis=1,
    )  # [64, 8192]
    return np.ascontiguousarray(out)


# revision 6
# speedup vs baseline: 1.0430x; 1.0430x over previous
"""Int4-packed linear (group-quantized, 256-group) on 8 Trainium2 cores.

Column-parallel: each core owns 1024 of 8192 out_features.

Math per core (out^T orientation, o on partitions):
  out[o, t] = sum_g s[o,g] * R_g[o,t] + corr[o,t]
  R_g[o,t]  = sum_{i in g} q[o,i] * x[t,i]        (q in 0..15)
  corr[o,t] = -8*sum_g s[o,g]*xsum_g[t] + bias[o]   (precomputed on host)

Weights ship as fp8e4m3 nibble planes (exact small integers), x as bf16.
The weight stream (8 MiB/core, o-tile-major contiguous reads) rides the
Sync HWDGE ring; scales/corr + x + output stores ride the Scalar HWDGE
ring so the two rings overlap. The stream runs at the per-core HBM
roofline, so the kernel is stream-bound in the middle. Stream order is
chosen to shorten the post-stream tail: o-tile 7's first 16 groups go
FIRST (combined early), and its last 12/3/1 groups arrive last as
shrinking pieces with a minimal DVE-only chain after the final byte.

Combine per o-tile, split by measured engine rates (ACT is the cheapest
PSUM reader at ~0.87 ns/elem/partition; DVE SBUF bf16 ops hit 2x mode;
Pool tensor_tensor costs ~2 ns/e/p so it gets exactly one op):
  ACT : prod[o,t,0:16]  = PSUM_A[o,g,t]  (raw transposing copy)
  ACT : prod[o,t,16:32] = PSUM_B[o,g,t]
  DVE : scl[o,t,0:32]   = prod * s[o,g]  (one 2x-mode multiply)
  Pool: half[o,t,0:16]  = scl lo + scl hi
  ACT : half[o,t,16]    = corr (staged early, off-chain)
  DVE : y[o,t] = reduce_X(half[o,t,0:17])
"""

import sys

import numpy as np
import ml_dtypes

sys.path.insert(0, "/opt/trn_rl_repo")

import concourse.bass as bass  # noqa: E402
import concourse.mybir as mybir  # noqa: E402
import concourse.tile as tile  # noqa: E402
from concourse import bacc  # noqa: E402

NCORES = 8
TOKENS = 64
IN_F = 8192
OUT_F = 8192
GROUP = 256
OC = OUT_F // NCORES  # 1024 out-features per core
NCHUNK = IN_F // 128  # 64 K-chunks of 128
NG = IN_F // GROUP  # 32 groups
NH = NG // 2  # 16 groups per PSUM half tile
NOT = OC // 128  # 8 o-tiles per core

_cache = {}


def _build_nc():
    if "nc" in _cache:
        return _cache["nc"], _cache["names"]

    f32 = mybir.dt.float32
    bf16 = mybir.dt.bfloat16
    nc = bacc.Bacc(None, target_bir_lowering=False, debug=False)
    with tile.TileContext(nc) as tc:
        with tc.tile_pool(name="dram", bufs=1, space="DRAM") as dram:
            # w8[ot, p, r, c] = nibble_fp8[ot*128 + c, 128*r + p]
            # (o-tile-major: each o-tile's weights are one contiguous 1 MiB)
            w8 = dram.tile([NOT, 128, NCHUNK, 128], mybir.dt.float8e4,
                           kind="ExternalInput")
            xt = dram.tile([128, NCHUNK, TOKENS], bf16, kind="ExternalInput")
            # scco[:, ot, 0:32] = scales, scco[:, ot, 32:96] = corr
            scco = dram.tile([128, NOT, NG + TOKENS], bf16,
                             kind="ExternalInput")
            outT = dram.tile([128, NOT, TOKENS], bf16, kind="ExternalOutput")

            with (
                tc.tile_pool(name="wsb", bufs=1) as wsb,
                tc.tile_pool(name="xsb", bufs=1) as xsb,
                tc.tile_pool(name="small", bufs=1) as small,
                tc.tile_pool(name="cmb", bufs=1) as cmb,
                tc.tile_pool(name="ps", bufs=4, space="PSUM") as ps,
            ):
                w_all = wsb.tile([128, NOT, NCHUNK, 128], mybir.dt.float8e4)
                x_all = xsb.tile([128, NCHUNK, TOKENS], bf16)
                scco_sb = small.tile([128, NOT, NG + TOKENS], bf16)
                prod_all = cmb.tile([128, NOT - 1, TOKENS, NG], bf16)
                scl_all = cmb.tile([128, NOT - 1, TOKENS, NG], bf16,
                                   tag="scl")
                half_all = cmb.tile([128, NOT - 1, TOKENS, NH + 1], bf16,
                                    tag="hf")
                # o-tile 7 staging: A half early, B/C/D pieces at the tail
                p7rawA = cmb.tile([128, TOKENS, 16], bf16, tag="p7a")
                comb7 = cmb.tile([128, TOKENS, 17], bf16, tag="c7")
                p7rawB = cmb.tile([128, TOKENS, 12], bf16, tag="p7b")
                comb7b = cmb.tile([128, TOKENS, 12], bf16, tag="c7b")
                c3 = cmb.tile([128, TOKENS, 3], bf16, tag="c3")
                d1 = cmb.tile([128, TOKENS, 1], bf16, tag="d1")
                part7 = cmb.tile([128, 4, TOKENS], bf16, tag="part")
                y_all = cmb.tile([128, NOT, TOKENS], bf16, tag="y")

                # scco + x on the Scalar HWDGE ring; the weight stream owns
                # the Sync ring. Both rings drain through the same 16 SDMA
                # engines, so aggregate rate caps at the per-core HBM limit
                # while per-ring receipt latency overlaps.
                nc.scalar.dma_start(out=scco_sb[:], in_=scco[:])
                nc.scalar.dma_start(out=x_all[:], in_=xt[:])

                # o-tile 7's A half leads the stream; its B/C/D pieces trail
                nc.sync.dma_start(out=w_all[:, NOT - 1, 0:32, :],
                                  in_=w8[NOT - 1, :, 0:32, :])
                for ot in range(NOT - 1):
                    nc.sync.dma_start(out=w_all[:, ot, :, :], in_=w8[ot])
                for c0, c1 in ((32, 56), (56, 62), (62, 64)):
                    nc.sync.dma_start(out=w_all[:, NOT - 1, c0:c1, :],
                                      in_=w8[NOT - 1, :, c0:c1, :])

                def mm_groups(ot, glo, ghi, r_ps):
                    # start once per 2 KB PSUM bank (8 slots of 64 fp32);
                    # the bank-wide pending-zero covers every slot's first
                    # write. stop on the last matmul touching each bank.
                    n = ghi - glo
                    for gg in range(n):
                        g = glo + gg
                        nc.tensor.matmul(
                            r_ps[:, gg, :],
                            lhsT=w_all[:, ot, 2 * g, :],
                            rhs=x_all[:, 2 * g, :],
                            start=(gg % 8 == 0),
                            stop=False,
                        )
                        nc.tensor.matmul(
                            r_ps[:, gg, :],
                            lhsT=w_all[:, ot, 2 * g + 1, :],
                            rhs=x_all[:, 2 * g + 1, :],
                            start=False,
                            stop=(gg == n - 1 or gg % 8 == 7),
                        )

                def psum_tg(r_ps, npg):
                    # PSUM [o, g, t] viewed as [o, t, g] (strided g-read,
                    # contiguous SBUF write)
                    return bass.AP(
                        tensor=r_ps.tensor,
                        offset=r_ps.offset,
                        ap=[r_ps.ap[0], [1, TOKENS], [TOKENS, npg]],
                    )

                def s_bc(ot, lo, n):
                    # s[o, g] broadcast along t
                    s_ot = scco_sb[:, ot, :]
                    return bass.AP(
                        tensor=s_ot.tensor,
                        offset=s_ot.offset + lo,
                        ap=[s_ot.ap[0], [0, TOKENS], [1, n]],
                    )

                mul = mybir.AluOpType.mult
                add = mybir.AluOpType.add
                X = mybir.AxisListType.X
                L7 = NOT - 1

                with nc.allow_low_precision("bf16 combine, validated vs 2e-2"):
                    # stage corr early (ACT, off the combine chain)
                    nc.scalar.copy(out=comb7[:, :, 16],
                                   in_=scco_sb[:, L7, NG:])
                    for ot in range(NOT - 1):
                        nc.scalar.copy(out=half_all[:, ot, :, NH],
                                       in_=scco_sb[:, ot, NG:])

                    # ---- o-tile 7 piece A (g0-15): early ----
                    rA7 = ps.tile([128, NH, TOKENS], f32, tag="rA", bufs=2)
                    mm_groups(L7, 0, 16, rA7)
                    nc.scalar.copy(out=p7rawA[:], in_=psum_tg(rA7, 16))
                    nc.vector.tensor_tensor(
                        out=comb7[:, :, 0:16], in0=p7rawA[:],
                        in1=s_bc(L7, 0, 16), op=mul)
                    nc.vector.tensor_reduce(
                        out=part7[:, 0, :], in_=comb7, axis=X, op=add)

                    # ---- o-tiles 0-6 ----
                    for ot in range(NOT - 1):
                        rA = ps.tile([128, NH, TOKENS], f32, tag="rA", bufs=2)
                        mm_groups(ot, 0, NH, rA)
                        rB = ps.tile([128, NH, TOKENS], f32, tag="rB", bufs=2)
                        mm_groups(ot, NH, NG, rB)
                        prod = prod_all[:, ot, :, :]
                        scl = scl_all[:, ot, :, :]
                        half = half_all[:, ot, :, :]
                        nc.scalar.copy(out=prod[:, :, 0:NH],
                                       in_=psum_tg(rA, NH))
                        nc.scalar.copy(out=prod[:, :, NH:NG],
                                       in_=psum_tg(rB, NH))
                        nc.vector.tensor_tensor(
                            out=scl[:], in0=prod[:], in1=s_bc(ot, 0, NG),
                            op=mul)
                        nc.gpsimd.tensor_tensor(
                            out=half[:, :, 0:NH], in0=scl[:, :, 0:NH],
                            in1=scl[:, :, NH:NG], op=add)
                        nc.vector.tensor_reduce(
                            out=y_all[:, ot, :], in_=half, axis=X, op=add)

                    # ship o-tiles 0-6 as one store on the Scalar ring
                    nc.scalar.dma_start(out=outT[:, 0:NOT - 1, :],
                                        in_=y_all[:, 0:NOT - 1, :])

                    # ---- o-tile 7 pieces B (g16-27), C (g28-30), D (g31) ----
                    rB7 = ps.tile([128, 12, TOKENS], f32, tag="rB", bufs=2)
                    mm_groups(L7, 16, 28, rB7)
                    nc.scalar.copy(out=p7rawB[:], in_=psum_tg(rB7, 12))
                    nc.vector.tensor_tensor(
                        out=comb7b[:], in0=p7rawB[:],
                        in1=s_bc(L7, 16, 12), op=mul)
                    nc.vector.tensor_reduce(
                        out=part7[:, 1, :], in_=comb7b, axis=X, op=add)
                    nc.vector.tensor_tensor(
                        out=part7[:, 2, :], in0=part7[:, 0, :],
                        in1=part7[:, 1, :], op=add)

                    rC7 = ps.tile([128, 3, TOKENS], f32, tag="rA", bufs=2)
                    mm_groups(L7, 28, 31, rC7)
                    nc.vector.tensor_tensor(
                        out=c3[:], in0=psum_tg(rC7, 3),
                        in1=s_bc(L7, 28, 3), op=mul)
                    nc.vector.tensor_reduce(
                        out=part7[:, 1, :], in_=c3, axis=X, op=add)
                    nc.vector.tensor_tensor(
                        out=part7[:, 3, :], in0=part7[:, 2, :],
                        in1=part7[:, 1, :], op=add)

                    rD7 = ps.tile([128, 1, TOKENS], f32, tag="rB", bufs=2)
                    mm_groups(L7, 31, 32, rD7)
                    nc.vector.tensor_tensor(
                        out=d1[:], in0=psum_tg(rD7, 1),
                        in1=s_bc(L7, 31, 1), op=mul)
                    for t0, t1 in ((0, TOKENS // 2), (TOKENS // 2, TOKENS)):
                        nc.vector.tensor_tensor(
                            out=y_all[:, L7, t0:t1],
                            in0=part7[:, 3, t0:t1],
                            in1=d1[:, t0:t1, 0], op=add)
                        nc.scalar.dma_start(out=outT[:, L7, t0:t1],
                                            in_=y_all[:, L7, t0:t1])

    nc.compile()
    names = dict(w8=w8.name, xt=xt.name, scco=scco.name, outT=outT.name)
    _cache["nc"] = nc
    _cache["names"] = names
    return nc, names


def _gather_core(outT_host):
    # outT_host[p, ot, t] -> [t, ot*128 + p]
    o = np.asarray(outT_host).astype(np.float32)
    return o.transpose(2, 1, 0).reshape(TOKENS, OC)


def _host_prep(x, weight_packed, scales, bias):
    """Build the 8 per-core input maps."""
    _, names = _build_nc()

    wp = np.ascontiguousarray(weight_packed).view(np.uint32)  # [8192, 1024]
    shifts = (np.arange(8, dtype=np.uint32) * 4)[None, None, :]
    nib = ((wp[:, :, None] >> shifts) & np.uint32(0xF)).astype(np.uint8)
    nib = nib.reshape(OUT_F, IN_F)  # n[o, i]
    lut = np.arange(16, dtype=np.float32).astype(ml_dtypes.float8_e4m3)
    nfp8 = lut[nib]  # [8192, 8192] fp8, exact

    xb = x.astype(ml_dtypes.bfloat16)
    xf = xb.astype(np.float32)
    # xt_host[p, r, t] = x_bf16[t, 128r + p]
    xt_host = np.ascontiguousarray(
        xb.T.reshape(NCHUNK, 128, TOKENS).transpose(1, 0, 2))
    # corr[o, t] = -8 * sum_g s[o,g] * xsum_g[t] + bias[o]
    xsum = xf.reshape(TOKENS, NG, GROUP).sum(axis=2)  # [t, g]
    corr = (-8.0 * scales.astype(np.float64) @ xsum.astype(np.float64).T
            + bias.astype(np.float64)[:, None]).astype(np.float32)  # [8192, 64]

    in_maps = []
    for k in range(NCORES):
        osl = slice(OC * k, OC * (k + 1))
        nk = nfp8[osl]  # [1024, 8192]
        # w8_host[ot, p, r, c] = nk[ot*128 + c, 128*r + p]
        w8_host = np.ascontiguousarray(
            nk.reshape(NOT, 128, NCHUNK, 128).transpose(0, 3, 2, 1)
        )
        sck = scales[osl]  # [1024, 32]
        scco_host = np.empty((128, NOT, NG + TOKENS), dtype=ml_dtypes.bfloat16)
        scco_host[:, :, :NG] = sck.reshape(NOT, 128, NG).transpose(1, 0, 2)
        scco_host[:, :, NG:] = corr[osl].reshape(NOT, 128, TOKENS).transpose(
            1, 0, 2)
        in_maps.append({
            names["w8"]: w8_host,
            names["xt"]: xt_host,
            names["scco"]: np.ascontiguousarray(scco_host),
        })
    return in_maps


def kernel(x, weight_packed, scales, bias):
    from concourse.bass_utils import run_bass_kernel_spmd

    nc, names = _build_nc()
    in_maps = _host_prep(x, weight_packed, scales, bias)
    res = run_bass_kernel_spmd(nc, in_maps, core_ids=list(range(NCORES)))
    out = np.concatenate(
        [_gather_core(res.results[k][names["outT"]]) for k in range(NCORES)],
        axis=1,
    )  # [64, 8192]
    return np.ascontiguousarray(out)


# revision 8
# speedup vs baseline: 1.0626x; 1.0188x over previous
"""Int4-packed linear (group-quantized, 256-group) on 8 Trainium2 cores.

Column-parallel: each core owns 1024 of 8192 out_features.

Math per core (out^T orientation, o on partitions):
  out[o, t] = sum_g s[o,g] * R_g[o,t] + corr[o,t]
  R_g[o,t]  = sum_{i in g} q[o,i] * x[t,i]        (q in 0..15)
  corr[o,t] = -8*sum_g s[o,g]*xsum_g[t] + bias[o]   (precomputed on host)

Weights ship as fp8e4m3 nibble planes (exact small integers), x as bf16.
x + weights stream on the Sync HWDGE ring at the per-core HBM roofline;
scco rides the Scalar ring. Stream order is chosen so the LAST-arriving
weights need only short DVE-only chains: o-tile 7's g0-15 and o-tile
6's g0-27 go early (their partials reduce mid-stream), and the tail is
three small pieces (6S=4g, 7B=12g, 7E=4g) plus tile 5.

Combine per full tile (0-4), split by measured engine rates (ACT is
the cheapest PSUM reader ~1.09 ns/e/p; DVE SBUF bf16 ops hit 2x mode
~0.6; Pool costs ~2.1 so it gets exactly one op; reduce is DVE-only):
  ACT : raw[o,t,0:32]  = PSUM_A|PSUM_B (2 transposing copies)
  DVE : scl[o,t,0:32]  = raw * s[o,g]  (one 2x-mode multiply)
  Pool: half[o,t,0:16] = scl lo + scl hi
  Pool: half[o,t,16]   = corr (staged early, off-chain)
  DVE : y[o,t] = reduce_X(half[o,t,0:17])
The DVE queue is software-pipelined (reduce_{k-1} issued after
scale_k) so it never head-of-line blocks on Pool's fold. Late pieces
skip ACT/Pool entirely (DVE mult straight from PSUM -> reduce -> add).
Output stores ride the Sync ring (SP is idle once the stream ends).
"""

import sys

import numpy as np
import ml_dtypes

sys.path.insert(0, "/opt/trn_rl_repo")

import concourse.bass as bass  # noqa: E402
import concourse.mybir as mybir  # noqa: E402
import concourse.tile as tile  # noqa: E402
from concourse import bacc  # noqa: E402

NCORES = 8
TOKENS = 64
IN_F = 8192
OUT_F = 8192
GROUP = 256
OC = OUT_F // NCORES  # 1024 out-features per core
NCHUNK = IN_F // 128  # 64 K-chunks of 128
NG = IN_F // GROUP  # 32 groups
NH = NG // 2  # 16 groups per PSUM half tile
NOT = OC // 128  # 8 o-tiles per core

_cache = {}


def _build_nc():
    if "nc" in _cache:
        return _cache["nc"], _cache["names"]

    f32 = mybir.dt.float32
    bf16 = mybir.dt.bfloat16
    nc = bacc.Bacc(None, target_bir_lowering=False, debug=False)
    with tile.TileContext(nc) as tc:
        with tc.tile_pool(name="dram", bufs=1, space="DRAM") as dram:
            # w8[ot, p, r, c] = nibble_fp8[ot*128 + c, 128*r + p]
            # (o-tile-major: each o-tile's weights are one contiguous 1 MiB)
            w8 = dram.tile([NOT, 128, NCHUNK, 128], mybir.dt.float8e4,
                           kind="ExternalInput")
            xt = dram.tile([128, NCHUNK, TOKENS], bf16, kind="ExternalInput")
            # scco[:, ot, 0:32] = scales, scco[:, ot, 32:96] = corr
            scco = dram.tile([128, NOT, NG + TOKENS], bf16,
                             kind="ExternalInput")
            outT = dram.tile([128, NOT, TOKENS], bf16, kind="ExternalOutput")

            with (
                tc.tile_pool(name="wsb", bufs=1) as wsb,
                tc.tile_pool(name="xsb", bufs=1) as xsb,
                tc.tile_pool(name="small", bufs=1) as small,
                tc.tile_pool(name="cmb", bufs=1) as cmb,
                tc.tile_pool(name="ps", bufs=2, space="PSUM") as ps,
            ):
                NF = 5  # tiles 0-4 use the full ACT/DVE/Pool pipeline
                w_all = wsb.tile([128, NOT, NCHUNK, 128], mybir.dt.float8e4)
                x_all = xsb.tile([128, NCHUNK, TOKENS], bf16)
                scco_sb = small.tile([128, NOT, NG + TOKENS], bf16)
                raw_all = cmb.tile([128, NF, TOKENS, NG], bf16)
                scl_all = cmb.tile([128, NF + 1, TOKENS, NG], bf16, tag="scl")
                half_all = cmb.tile([128, NF + 1, TOKENS, NH + 1], bf16,
                                    tag="hf")
                # o-tile 7: A half (16g+corr) early, B (12g) + E (4g) late
                rawA7 = cmb.tile([128, TOKENS, 16], bf16, tag="p7a")
                comb7 = cmb.tile([128, TOKENS, 17], bf16, tag="c7")
                rawB7 = cmb.tile([128, TOKENS, 12], bf16, tag="p7b")
                sclB7 = cmb.tile([128, TOKENS, 12], bf16, tag="c7b")
                sclE7 = cmb.tile([128, TOKENS, 4], bf16, tag="e7")
                # o-tile 6: g0-27 early (reduced with corr), g28-31 late
                raw6 = cmb.tile([128, TOKENS, 28], bf16, tag="r6")
                scl6 = cmb.tile([128, TOKENS, 29], bf16, tag="s6")
                sclS6 = cmb.tile([128, TOKENS, 4], bf16, tag="s6s")
                part = cmb.tile([128, 6, TOKENS], bf16, tag="part")
                y_all = cmb.tile([128, NOT, TOKENS], bf16, tag="y")

                # scco on the Scalar ring (tiny); x + weights own the Sync
                # ring in consumption order.
                nc.scalar.dma_start(out=scco_sb[:], in_=scco[:])

                nc.sync.dma_start(out=x_all[:], in_=xt[:])
                nc.sync.dma_start(out=w_all[:, 7, 0:32, :],
                                  in_=w8[7, :, 0:32, :])  # 7A g0-15
                nc.sync.dma_start(out=w_all[:, 6, 0:56, :],
                                  in_=w8[6, :, 0:56, :])  # 6M g0-27
                for ot in range(NF):
                    nc.sync.dma_start(out=w_all[:, ot, :, :], in_=w8[ot])
                nc.sync.dma_start(out=w_all[:, 5, 0:32, :],
                                  in_=w8[5, :, 0:32, :])  # t5 A half
                nc.sync.dma_start(out=w_all[:, 5, 32:64, :],
                                  in_=w8[5, :, 32:64, :])  # t5 B half
                nc.sync.dma_start(out=w_all[:, 6, 56:64, :],
                                  in_=w8[6, :, 56:64, :])  # 6S g28-31
                nc.sync.dma_start(out=w_all[:, 7, 32:56, :],
                                  in_=w8[7, :, 32:56, :])  # 7B g16-27
                nc.sync.dma_start(out=w_all[:, 7, 56:64, :],
                                  in_=w8[7, :, 56:64, :])  # 7E g28-31

                def mm_groups(ot, glo, ghi, r_ps):
                    # start once per 2 KB PSUM bank (8 slots of 64 fp32);
                    # the bank-wide pending-zero covers every slot's first
                    # write. stop on the last matmul touching each bank.
                    n = ghi - glo
                    for gg in range(n):
                        g = glo + gg
                        nc.tensor.matmul(
                            r_ps[:, gg, :],
                            lhsT=w_all[:, ot, 2 * g, :],
                            rhs=x_all[:, 2 * g, :],
                            start=(gg % 8 == 0),
                            stop=False,
                        )
                        nc.tensor.matmul(
                            r_ps[:, gg, :],
                            lhsT=w_all[:, ot, 2 * g + 1, :],
                            rhs=x_all[:, 2 * g + 1, :],
                            start=False,
                            stop=(gg == n - 1 or gg % 8 == 7),
                        )

                def psum_tg(r_ps, npg):
                    # PSUM [o, g, t] viewed as [o, t, g] (strided g-read,
                    # contiguous SBUF write)
                    return bass.AP(
                        tensor=r_ps.tensor,
                        offset=r_ps.offset,
                        ap=[r_ps.ap[0], [1, TOKENS], [TOKENS, npg]],
                    )

                def s_bc(ot, lo, n):
                    # s[o, g] broadcast along t
                    s_ot = scco_sb[:, ot, :]
                    return bass.AP(
                        tensor=s_ot.tensor,
                        offset=s_ot.offset + lo,
                        ap=[s_ot.ap[0], [0, TOKENS], [1, n]],
                    )

                mul = mybir.AluOpType.mult
                add = mybir.AluOpType.add
                X = mybir.AxisListType.X

                def mkps(n, tag):
                    return ps.tile([128, n, TOKENS], f32, tag=tag, bufs=2,
                                   name=tag)

                with nc.allow_low_precision("bf16 combine, validated vs 2e-2"):
                    # corr staging (Pool, early, off-chain)
                    nc.gpsimd.tensor_copy(out=comb7[:, :, 16],
                                          in_=scco_sb[:, 7, NG:])
                    nc.gpsimd.tensor_copy(out=scl6[:, :, 28],
                                          in_=scco_sb[:, 6, NG:])
                    for ot in range(NF):
                        nc.gpsimd.tensor_copy(out=half_all[:, ot, :, NH],
                                              in_=scco_sb[:, ot, NG:])
                    nc.gpsimd.tensor_copy(out=half_all[:, 5, :, NH],
                                          in_=scco_sb[:, 5, NG:])

                    # ---- 7A (g0-15) early: reduce with corr -> part[0] ----
                    rA7 = mkps(NH, "rA")
                    mm_groups(7, 0, 16, rA7)
                    nc.scalar.copy(out=rawA7[:], in_=psum_tg(rA7, 16))
                    nc.vector.tensor_tensor(
                        out=comb7[:, :, 0:16], in0=rawA7[:],
                        in1=s_bc(7, 0, 16), op=mul)
                    nc.vector.tensor_reduce(
                        out=part[:, 0, :], in_=comb7, axis=X, op=add)

                    # ---- 6M (g0-27) early: reduce with corr -> part[1] ----
                    rA6 = mkps(NH, "rA")
                    mm_groups(6, 0, 16, rA6)
                    r6m = mkps(12, "rB")
                    mm_groups(6, 16, 28, r6m)
                    nc.scalar.copy(out=raw6[:, :, 0:16], in_=psum_tg(rA6, 16))
                    nc.scalar.copy(out=raw6[:, :, 16:28],
                                   in_=psum_tg(r6m, 12))
                    nc.vector.tensor_tensor(
                        out=scl6[:, :, 0:28], in0=raw6[:],
                        in1=s_bc(6, 0, 28), op=mul)
                    nc.vector.tensor_reduce(
                        out=part[:, 1, :], in_=scl6, axis=X, op=add)

                    # ---- tiles 0-4: full pipeline, DVE skewed ----
                    reduces = []  # deferred (ot, emit_fn) for the skew

                    def emit_reduce(ot):
                        nc.vector.tensor_reduce(
                            out=y_all[:, ot, :], in_=half_all[:, ot, :, :],
                            axis=X, op=add)

                    for ot in range(NF):
                        rA = mkps(NH, "rA")
                        mm_groups(ot, 0, NH, rA)
                        rB = mkps(NH, "rB")
                        mm_groups(ot, NH, NG, rB)
                        raw = raw_all[:, ot, :, :]
                        scl = scl_all[:, ot, :, :]
                        half = half_all[:, ot, :, :]
                        nc.scalar.copy(out=raw[:, :, 0:NH],
                                       in_=psum_tg(rA, NH))
                        nc.scalar.copy(out=raw[:, :, NH:NG],
                                       in_=psum_tg(rB, NH))
                        nc.vector.tensor_tensor(
                            out=scl[:], in0=raw[:], in1=s_bc(ot, 0, NG),
                            op=mul)
                        nc.gpsimd.tensor_tensor(
                            out=half[:, :, 0:NH], in0=scl[:, :, 0:NH],
                            in1=scl[:, :, NH:NG], op=add)
                        if ot >= 1:
                            emit_reduce(ot - 1)  # skew: never blocks on Pool

                    emit_reduce(NF - 1)
                    # ship tiles 0-4 as one store on the (now idle) Sync ring
                    nc.sync.dma_start(out=outT[:, 0:NF, :],
                                      in_=y_all[:, 0:NF, :])

                    # ---- tile 5: DVE-direct (no ACT round-trip) ----
                    rA5 = mkps(NH, "rA")
                    mm_groups(5, 0, NH, rA5)
                    rB5 = mkps(NH, "rB")
                    mm_groups(5, NH, NG, rB5)
                    scl5 = scl_all[:, 5, :, :]
                    half5 = half_all[:, 5, :, :]
                    nc.vector.tensor_tensor(
                        out=scl5[:, :, 0:NH], in0=psum_tg(rA5, NH),
                        in1=s_bc(5, 0, NH), op=mul)

                    # ---- 6S (g28-31): DVE-direct -> y6 ----
                    rS6 = mkps(4, "rA")
                    mm_groups(6, 28, 32, rS6)
                    nc.vector.tensor_tensor(
                        out=sclS6[:], in0=psum_tg(rS6, 4),
                        in1=s_bc(6, 28, 4), op=mul)
                    nc.vector.tensor_reduce(
                        out=part[:, 2, :], in_=sclS6, axis=X, op=add)
                    nc.vector.tensor_tensor(
                        out=y_all[:, 6, :], in0=part[:, 1, :],
                        in1=part[:, 2, :], op=add)

                    # tile 5 continues: B half + fold + reduce on DVE
                    nc.vector.tensor_tensor(
                        out=scl5[:, :, NH:NG], in0=psum_tg(rB5, NH),
                        in1=s_bc(5, NH, NH), op=mul)
                    nc.vector.tensor_tensor(
                        out=half5[:, :, 0:NH], in0=scl5[:, :, 0:NH],
                        in1=scl5[:, :, NH:NG], op=add)

                    # ---- 7B (g16-27): ACT copy (ACT is idle) + DVE ----
                    rB7 = mkps(12, "rB")
                    mm_groups(7, 16, 28, rB7)
                    nc.scalar.copy(out=rawB7[:], in_=psum_tg(rB7, 12))
                    nc.vector.tensor_tensor(
                        out=sclB7[:], in0=rawB7[:], in1=s_bc(7, 16, 12),
                        op=mul)
                    nc.vector.tensor_reduce(
                        out=part[:, 3, :], in_=sclB7, axis=X, op=add)
                    nc.vector.tensor_tensor(
                        out=part[:, 4, :], in0=part[:, 0, :],
                        in1=part[:, 3, :], op=add)

                    # tile 5 reduce + y56 store
                    nc.vector.tensor_reduce(
                        out=y_all[:, 5, :], in_=half5, axis=X, op=add)
                    nc.sync.dma_start(out=outT[:, 5:7, :],
                                      in_=y_all[:, 5:7, :])

                    # ---- 7E (g28-31): DVE-direct, token-split finish ----
                    rE7 = mkps(4, "rA")
                    mm_groups(7, 28, 32, rE7)
                    nc.vector.tensor_tensor(
                        out=sclE7[:], in0=psum_tg(rE7, 4),
                        in1=s_bc(7, 28, 4), op=mul)
                    nc.vector.tensor_reduce(
                        out=part[:, 5, :], in_=sclE7, axis=X, op=add)
                    for t0, t1 in ((0, TOKENS // 2), (TOKENS // 2, TOKENS)):
                        nc.vector.tensor_tensor(
                            out=y_all[:, 7, t0:t1],
                            in0=part[:, 4, t0:t1],
                            in1=part[:, 5, t0:t1], op=add)
                        nc.sync.dma_start(out=outT[:, 7, t0:t1],
                                          in_=y_all[:, 7, t0:t1])

    nc.compile()
    names = dict(w8=w8.name, xt=xt.name, scco=scco.name, outT=outT.name)
    _cache["nc"] = nc
    _cache["names"] = names
    return nc, names


def _gather_core(outT_host):
    # outT_host[p, ot, t] -> [t, ot*128 + p]
    o = np.asarray(outT_host).astype(np.float32)
    return o.transpose(2, 1, 0).reshape(TOKENS, OC)


def _host_prep(x, weight_packed, scales, bias):
    """Build the 8 per-core input maps."""
    _, names = _build_nc()

    wp = np.ascontiguousarray(weight_packed).view(np.uint32)  # [8192, 1024]
    shifts = (np.arange(8, dtype=np.uint32) * 4)[None, None, :]
    nib = ((wp[:, :, None] >> shifts) & np.uint32(0xF)).astype(np.uint8)
    nib = nib.reshape(OUT_F, IN_F)  # n[o, i]
    lut = np.arange(16, dtype=np.float32).astype(ml_dtypes.float8_e4m3)
    nfp8 = lut[nib]  # [8192, 8192] fp8, exact

    xb = x.astype(ml_dtypes.bfloat16)
    xf = xb.astype(np.float32)
    # xt_host[p, r, t] = x_bf16[t, 128r + p]
    xt_host = np.ascontiguousarray(
        xb.T.reshape(NCHUNK, 128, TOKENS).transpose(1, 0, 2))
    # corr[o, t] = -8 * sum_g s[o,g] * xsum_g[t] + bias[o]
    xsum = xf.reshape(TOKENS, NG, GROUP).sum(axis=2)  # [t, g]
    corr = (-8.0 * scales.astype(np.float64) @ xsum.astype(np.float64).T
            + bias.astype(np.float64)[:, None]).astype(np.float32)  # [8192, 64]

    in_maps = []
    for k in range(NCORES):
        osl = slice(OC * k, OC * (k + 1))
        nk = nfp8[osl]  # [1024, 8192]
        # w8_host[ot, p, r, c] = nk[ot*128 + c, 128*r + p]
        w8_host = np.ascontiguousarray(
            nk.reshape(NOT, 128, NCHUNK, 128).transpose(0, 3, 2, 1)
        )
        sck = scales[osl]  # [1024, 32]
        scco_host = np.empty((128, NOT, NG + TOKENS), dtype=ml_dtypes.bfloat16)
        scco_host[:, :, :NG] = sck.reshape(NOT, 128, NG).transpose(1, 0, 2)
        scco_host[:, :, NG:] = corr[osl].reshape(NOT, 128, TOKENS).transpose(
            1, 0, 2)
        in_maps.append({
            names["w8"]: w8_host,
            names["xt"]: xt_host,
            names["scco"]: np.ascontiguousarray(scco_host),
        })
    return in_maps


def kernel(x, weight_packed, scales, bias):
    from concourse.bass_utils import run_bass_kernel_spmd

    nc, names = _build_nc()
    in_maps = _host_prep(x, weight_packed, scales, bias)
    res = run_bass_kernel_spmd(nc, in_maps, core_ids=list(range(NCORES)))
    out = np.concatenate(
        [_gather_core(res.results[k][names["outT"]]) for k in range(NCORES)],
        axis=1,
    )  # [64, 8192]
    return np.ascontiguousarray(out)


# revision 9
# speedup vs baseline: 1.1518x; 1.0839x over previous
"""Int4-packed linear (group-quantized, 256-group) on 8 Trainium2 cores.

Column-parallel: each core owns 1024 of 8192 out_features.

Math per core (out^T orientation, o on partitions):
  out[o, t] = sum_g s[o,g] * R_g[o,t] + corr[o,t]
  R_g[o,t]  = sum_{i in g} q[o,i] * x[t,i]        (q in 0..15)
  corr[o,t] = -8*sum_g s[o,g]*xsum_g[t] + bias[o]   (precomputed on host)

Weights ship as fp8e4m3 nibble planes (exact small integers), x as bf16.
Everything streams on the Sync HWDGE ring in consumption order (scco
and x first, weight tiles in half-tile units so matmuls/extraction
start ~1.3 us earlier per tile, o-tile 7 last in 16/8/8-group pieces
with a short DVE-only finish). Output stores also ride Sync (SP is
idle once the stream drains). The stream runs at the per-core HBM
roofline (~25 us); the exec time is set by DVE's total work, so the
combine is split to minimize DVE load:
  ACT : raw[o,t,g]     PSUM extraction, both halves (1.09 ns/e/p)
  DVE : scl = raw * s  one 2x-mode multiply per tile (0.6 ns/e/p)
  Pool: half = lo + hi fold (it is slow, ~2.05 ns/e/p, gets ONE op)
  Pool: corr staging (early, off-chain)
  DVE : y = reduce_X(half[o,t,0:17])  (reduce is DVE-only)
The static per-engine order is forced via tile_set_cur_wait stamps so
the scheduler cannot head-of-line block DVE's reduce_k behind Pool's
fold_k (its Pool cost model is optimistic); reduce_{k-1} is ordered
after scale_k. Tile 6's fold runs on DVE (Pool would be the tail).
"""

import sys

import numpy as np
import ml_dtypes

sys.path.insert(0, "/opt/trn_rl_repo")

import concourse.bass as bass  # noqa: E402
import concourse.mybir as mybir  # noqa: E402
import concourse.tile as tile  # noqa: E402
from concourse import bacc  # noqa: E402

NCORES = 8
TOKENS = 64
IN_F = 8192
OUT_F = 8192
GROUP = 256
OC = OUT_F // NCORES  # 1024 out-features per core
NCHUNK = IN_F // 128  # 64 K-chunks of 128
NG = IN_F // GROUP  # 32 groups
NH = NG // 2  # 16 groups per PSUM half tile
NOT = OC // 128  # 8 o-tiles per core

_cache = {}


def _build_nc():
    if "nc" in _cache:
        return _cache["nc"], _cache["names"]

    f32 = mybir.dt.float32
    bf16 = mybir.dt.bfloat16
    nc = bacc.Bacc(None, target_bir_lowering=False, debug=False)
    with tile.TileContext(nc) as tc:
        stamp_n = [0]

        def stamp():
            # Monotone scheduler-sim timestamps: forces the committed
            # per-engine static order to equal emission order.
            stamp_n[0] += 1
            tc.tile_set_cur_wait(0.002 * stamp_n[0])

        with tc.tile_pool(name="dram", bufs=1, space="DRAM") as dram:
            # w8[ot, p, r, c] = nibble_fp8[ot*128 + c, 128*r + p]
            w8 = dram.tile([NOT, 128, NCHUNK, 128], mybir.dt.float8e4,
                           kind="ExternalInput")
            xt = dram.tile([128, NCHUNK, TOKENS], bf16, kind="ExternalInput")
            # scco[:, ot, 0:32] = scales, scco[:, ot, 32:96] = corr
            scco = dram.tile([128, NOT, NG + TOKENS], bf16,
                             kind="ExternalInput")
            outT = dram.tile([128, NOT, TOKENS], bf16, kind="ExternalOutput")

            with (
                tc.tile_pool(name="wsb", bufs=1) as wsb,
                tc.tile_pool(name="xsb", bufs=1) as xsb,
                tc.tile_pool(name="small", bufs=1) as small,
                tc.tile_pool(name="cmb", bufs=1) as cmb,
                tc.tile_pool(name="ps", bufs=2, space="PSUM") as ps,
            ):
                NF = NOT - 1  # tiles 0-6 are full; o-tile 7 is pieced
                w_all = wsb.tile([128, NOT, NCHUNK, 128], mybir.dt.float8e4)
                x_all = xsb.tile([128, NCHUNK, TOKENS], bf16)
                scco_sb = small.tile([128, NOT, NG + TOKENS], bf16)
                raw_all = cmb.tile([128, NF, TOKENS, NG], bf16)
                scl_all = cmb.tile([128, NF, TOKENS, NG], bf16, tag="scl")
                half_all = cmb.tile([128, NF, TOKENS, NH + 1], bf16,
                                    tag="hf")
                # o-tile 7: A half reduced with corr; B half in two 8g bits
                rawA7 = cmb.tile([128, TOKENS, 16], bf16, tag="p7a")
                comb7 = cmb.tile([128, TOKENS, 17], bf16, tag="c7")
                rawB7 = cmb.tile([128, TOKENS, 16], bf16, tag="p7b")
                sclB7 = cmb.tile([128, TOKENS, 16], bf16, tag="s7b")
                part = cmb.tile([128, 4, TOKENS], bf16, tag="part")
                y_all = cmb.tile([128, NOT, TOKENS], bf16, tag="y")

                # Everything on the Sync ring, in consumption order.
                stamp()
                nc.sync.dma_start(out=scco_sb[:], in_=scco[:])
                stamp()
                nc.sync.dma_start(out=x_all[:], in_=xt[:])
                for ot in range(NF):
                    stamp()
                    nc.sync.dma_start(out=w_all[:, ot, 0:32, :],
                                      in_=w8[ot, :, 0:32, :])
                    stamp()
                    nc.sync.dma_start(out=w_all[:, ot, 32:64, :],
                                      in_=w8[ot, :, 32:64, :])
                for c0, c1 in ((0, 32), (32, 48), (48, 64)):
                    stamp()
                    nc.sync.dma_start(out=w_all[:, NOT - 1, c0:c1, :],
                                      in_=w8[NOT - 1, :, c0:c1, :])

                def mm_groups(ot, glo, ghi, r_ps):
                    # start once per 2 KB PSUM bank (8 slots of 64 fp32);
                    # the bank-wide pending-zero covers every slot's first
                    # write. stop on the last matmul touching each bank.
                    n = ghi - glo
                    for gg in range(n):
                        g = glo + gg
                        stamp()
                        nc.tensor.matmul(
                            r_ps[:, gg, :],
                            lhsT=w_all[:, ot, 2 * g, :],
                            rhs=x_all[:, 2 * g, :],
                            start=(gg % 8 == 0),
                            stop=False,
                        )
                        stamp()
                        nc.tensor.matmul(
                            r_ps[:, gg, :],
                            lhsT=w_all[:, ot, 2 * g + 1, :],
                            rhs=x_all[:, 2 * g + 1, :],
                            start=False,
                            stop=(gg == n - 1 or gg % 8 == 7),
                        )

                def psum_tg(r_ps, npg):
                    # PSUM [o, g, t] viewed as [o, t, g] (strided g-read,
                    # contiguous SBUF write)
                    return bass.AP(
                        tensor=r_ps.tensor,
                        offset=r_ps.offset,
                        ap=[r_ps.ap[0], [1, TOKENS], [TOKENS, npg]],
                    )

                def s_bc(ot, lo, n):
                    # s[o, g] broadcast along t
                    s_ot = scco_sb[:, ot, :]
                    return bass.AP(
                        tensor=s_ot.tensor,
                        offset=s_ot.offset + lo,
                        ap=[s_ot.ap[0], [0, TOKENS], [1, n]],
                    )

                mul = mybir.AluOpType.mult
                add = mybir.AluOpType.add
                X = mybir.AxisListType.X

                def mkps(n, tag):
                    return ps.tile([128, n, TOKENS], f32, tag=tag, bufs=2,
                                   name=tag)

                with nc.allow_low_precision("bf16 combine, validated vs 2e-2"):
                    # corr staging (Pool, early, off-chain)
                    for ot in range(NF):
                        stamp()
                        nc.gpsimd.tensor_copy(out=half_all[:, ot, :, NH],
                                              in_=scco_sb[:, ot, NG:])
                    stamp()
                    nc.gpsimd.tensor_copy(out=comb7[:, :, 16],
                                          in_=scco_sb[:, NOT - 1, NG:])

                    def emit_reduce(ot):
                        stamp()
                        nc.vector.tensor_reduce(
                            out=y_all[:, ot, :], in_=half_all[:, ot, :, :],
                            axis=X, op=add)

                    for ot in range(NF):
                        rA = mkps(NH, "rA")
                        mm_groups(ot, 0, NH, rA)
                        rB = mkps(NH, "rB")
                        mm_groups(ot, NH, NG, rB)
                        raw = raw_all[:, ot, :, :]
                        scl = scl_all[:, ot, :, :]
                        half = half_all[:, ot, :, :]
                        stamp()
                        nc.scalar.copy(out=raw[:, :, 0:NH],
                                       in_=psum_tg(rA, NH))
                        stamp()
                        nc.scalar.copy(out=raw[:, :, NH:NG],
                                       in_=psum_tg(rB, NH))
                        stamp()
                        nc.vector.tensor_tensor(
                            out=scl[:], in0=raw[:], in1=s_bc(ot, 0, NG),
                            op=mul)
                        stamp()
                        fold_engine = (nc.vector if ot == NF - 1
                                       else nc.gpsimd)
                        fold_engine.tensor_tensor(
                            out=half[:, :, 0:NH], in0=scl[:, :, 0:NH],
                            in1=scl[:, :, NH:NG], op=add)
                        if ot >= 1:
                            emit_reduce(ot - 1)  # skew vs Pool's fold

                    emit_reduce(NF - 2)  # reduce_5
                    stamp()
                    nc.sync.dma_start(out=outT[:, 0:NF - 1, :],
                                      in_=y_all[:, 0:NF - 1, :])

                    # ---- o-tile 7: A (g0-15) + corr, then B in two 8g ----
                    L7 = NOT - 1
                    rA7 = mkps(NH, "rA")
                    mm_groups(L7, 0, 16, rA7)
                    stamp()
                    nc.scalar.copy(out=rawA7[:], in_=psum_tg(rA7, 16))
                    stamp()
                    nc.vector.tensor_tensor(
                        out=comb7[:, :, 0:16], in0=rawA7[:],
                        in1=s_bc(L7, 0, 16), op=mul)
                    stamp()
                    nc.vector.tensor_reduce(
                        out=part[:, 0, :], in_=comb7, axis=X, op=add)
                    emit_reduce(NF - 1)  # reduce_6 (DVE fold_6 done above)
                    stamp()
                    nc.sync.dma_start(out=outT[:, NF - 1, :],
                                      in_=y_all[:, NF - 1, :])

                    rB1 = mkps(8, "rB")
                    mm_groups(L7, 16, 24, rB1)
                    stamp()
                    nc.scalar.copy(out=rawB7[:, :, 0:8], in_=psum_tg(rB1, 8))
                    stamp()
                    nc.vector.tensor_tensor(
                        out=sclB7[:, :, 0:8], in0=rawB7[:, :, 0:8],
                        in1=s_bc(L7, 16, 8), op=mul)
                    stamp()
                    nc.vector.tensor_reduce(
                        out=part[:, 1, :], in_=sclB7[:, :, 0:8], axis=X,
                        op=add)

                    rB2 = mkps(8, "rA")
                    mm_groups(L7, 24, 32, rB2)
                    stamp()
                    nc.scalar.copy(out=rawB7[:, :, 8:16],
                                   in_=psum_tg(rB2, 8))
                    stamp()
                    nc.vector.tensor_tensor(
                        out=sclB7[:, :, 8:16], in0=rawB7[:, :, 8:16],
                        in1=s_bc(L7, 24, 8), op=mul)
                    stamp()
                    nc.vector.tensor_reduce(
                        out=part[:, 2, :], in_=sclB7[:, :, 8:16], axis=X,
                        op=add)
                    stamp()
                    nc.vector.tensor_tensor(
                        out=part[:, 3, :], in0=part[:, 0, :],
                        in1=part[:, 1, :], op=add)
                    for t0, t1 in ((0, TOKENS // 2), (TOKENS // 2, TOKENS)):
                        stamp()
                        nc.vector.tensor_tensor(
                            out=y_all[:, L7, t0:t1],
                            in0=part[:, 3, t0:t1],
                            in1=part[:, 2, t0:t1], op=add)
                        stamp()
                        nc.sync.dma_start(out=outT[:, L7, t0:t1],
                                          in_=y_all[:, L7, t0:t1])

    nc.compile()
    names = dict(w8=w8.name, xt=xt.name, scco=scco.name, outT=outT.name)
    _cache["nc"] = nc
    _cache["names"] = names
    return nc, names


def _gather_core(outT_host):
    # outT_host[p, ot, t] -> [t, ot*128 + p]
    o = np.asarray(outT_host).astype(np.float32)
    return o.transpose(2, 1, 0).reshape(TOKENS, OC)


def _host_prep(x, weight_packed, scales, bias):
    """Build the 8 per-core input maps."""
    _, names = _build_nc()

    wp = np.ascontiguousarray(weight_packed).view(np.uint32)  # [8192, 1024]
    shifts = (np.arange(8, dtype=np.uint32) * 4)[None, None, :]
    nib = ((wp[:, :, None] >> shifts) & np.uint32(0xF)).astype(np.uint8)
    nib = nib.reshape(OUT_F, IN_F)  # n[o, i]
    lut = np.arange(16, dtype=np.float32).astype(ml_dtypes.float8_e4m3)
    nfp8 = lut[nib]  # [8192, 8192] fp8, exact

    xb = x.astype(ml_dtypes.bfloat16)
    xf = xb.astype(np.float32)
    # xt_host[p, r, t] = x_bf16[t, 128r + p]
    xt_host = np.ascontiguousarray(
        xb.T.reshape(NCHUNK, 128, TOKENS).transpose(1, 0, 2))
    # corr[o, t] = -8 * sum_g s[o,g] * xsum_g[t] + bias[o]
    xsum = xf.reshape(TOKENS, NG, GROUP).sum(axis=2)  # [t, g]
    corr = (-8.0 * scales.astype(np.float64) @ xsum.astype(np.float64).T
            + bias.astype(np.float64)[:, None]).astype(np.float32)  # [8192, 64]

    in_maps = []
    for k in range(NCORES):
        osl = slice(OC * k, OC * (k + 1))
        nk = nfp8[osl]  # [1024, 8192]
        # w8_host[ot, p, r, c] = nk[ot*128 + c, 128*r + p]
        w8_host = np.ascontiguousarray(
            nk.reshape(NOT, 128, NCHUNK, 128).transpose(0, 3, 2, 1)
        )
        sck = scales[osl]  # [1024, 32]
        scco_host = np.empty((128, NOT, NG + TOKENS), dtype=ml_dtypes.bfloat16)
        scco_host[:, :, :NG] = sck.reshape(NOT, 128, NG).transpose(1, 0, 2)
        scco_host[:, :, NG:] = corr[osl].reshape(NOT, 128, TOKENS).transpose(
            1, 0, 2)
        in_maps.append({
            names["w8"]: w8_host,
            names["xt"]: xt_host,
            names["scco"]: np.ascontiguousarray(scco_host),
        })
    return in_maps


def kernel(x, weight_packed, scales, bias):
    from concourse.bass_utils import run_bass_kernel_spmd

    nc, names = _build_nc()
    in_maps = _host_prep(x, weight_packed, scales, bias)
    res = run_bass_kernel_spmd(nc, in_maps, core_ids=list(range(NCORES)))
    out = np.concatenate(
        [_gather_core(res.results[k][names["outT"]]) for k in range(NCORES)],
        axis=1,
    )  # [64, 8192]
    return np.ascontiguousarray(out)
